# revision 23
# baseline (speedup 1.0000x reference)
"""Strided (stride=1) valid 1D conv on Trainium2, data-parallel over batch.

Problem: x (16, 32, 32768) f32, kernel (1, 32, 32, 3) f32
         -> out (16, 32, 32766) f32  (valid conv, NCH / OIH layout)

Strategy (per core, 2 batches each across 8 cores):
  out[b, co, l] = sum_{ci,k} W[co, ci, k] * x[b, ci, l + k]

  Channel count is 32, so we pack 4 independent L-chunks ("groups") into
  the 128 SBUF partitions: partition (g*32 + ci) holds x[b, ci, base+g*G+j].
  A block-diagonal [128, 128] weight matrix (4 copies of W_k^T on the
  diagonal) turns the 4-group conv tap into ONE K=128 matmul.  The 3 taps
  accumulate into one PSUM bank (start/stop flags).  Operands are typed
  float32r for the fast PE path (1 cycle/row at N=512 vs 4 for fp32).

  The host pre-packs x into the exact SBUF layout and unpacks the packed
  output, so every device DMA is one contiguous stream per partition row.
  Measured on HW: contiguous descriptors ~350-380 GB/s/core vs ~130 GB/s
  for 3D-strided ones; 4.2 MB transfers beat 2.1 MB by ~7%.  One in-DMA
  and one out-DMA per local batch ("pair" of two 16K-column tiles).

  Raw Bass (not Tile): walrus codegen in this toolchain embeds at most
  ONE sync wait per Matmult / HWDGE DMACopy, which Tile's auto-generated
  semaphores routinely exceed.  Every cross-engine wait here is an
  explicit standalone wait_ge on the engine's sequencer:
    sync   : weight DMA, then per-pair input DMAs interleaved with
             output DMAs (lagged one pair).
    tensor : per tile, 8 groups x 3 accumulating fp32r matmuls; PSUM
             bank j is recycled across tiles, gated on the drain of its
             previous occupant; last matmul of a group bumps sem_mm.
    scalar : drains PSUM->SBUF for even global tiles (ACT engine).
    vector : drains PSUM->SBUF for odd global tiles (DVE engine).

  reps > 1 repeats the whole pipeline in one NEFF (benchmarking only).
"""

import sys

if "/opt/trn_rl_repo" not in sys.path:
    sys.path.insert(0, "/opt/trn_rl_repo")

from contextlib import ExitStack

import numpy as np

import concourse.bass as bass
import concourse.mybir as mybir
from concourse.bass_utils import run_bass_kernel_spmd

# Problem shape (hardcoded; harness contract)
B, C, L = 16, 32, 32768
CO, KT = 32, 3
LOUT = L - KT + 1  # 32766
NCORES = 8
BPC = B // NCORES  # batches per core = 2

# Padded shapes
LP = L + 2  # x padded with 2 trailing zero columns
LOP = L     # output computed padded to 32768 (last 2 cols garbage)

# Tiling
NG = 4              # L-groups packed across the 128 partitions
G = 4096            # columns per group per tile
TILE_L = NG * G     # 16384 output cols per tile
NT = LOP // TILE_L  # tiles per batch = 2
NJ = G // 512       # 512-wide matmul chunks per group = 8
NTILES = BPC * NT   # 4 tiles/core; pair p (= local batch) owns tiles 2p,2p+1
XW = NT * (G + 2)   # xt pair-row width
OW = NT * G         # osb pair-row width

_CACHE = {}


def _cp_sem_count(gt: int, j: int) -> int:
    """Drain-engine sem value after copy (gt, j) completes.

    ACT drains even global tiles, DVE odd ones; each engine's sem counts
    its own copies in order.  gt = rep * NTILES + ti.
    """
    return NJ * (gt // 2) + j + 1


def _build_nc(reps: int = 1, act_out: bool = False):
    f32 = mybir.dt.float32
    f32r = mybir.dt.float32r

    nc = bass.Bass(trn_type="TRN2", target_bir_lowering=False)
    x = nc.dram_tensor("x", [BPC, 128, XW], f32r, kind="ExternalInput")
    w = nc.dram_tensor("w", [128, KT * 128], f32r, kind="ExternalInput")
    out = nc.dram_tensor("out", [BPC, 128, OW], f32, kind="ExternalOutput")

    with ExitStack() as ctx:
        wt = ctx.enter_context(nc.sbuf_tensor("wt", [128, KT * 128], f32r))
        xts = [
            ctx.enter_context(nc.sbuf_tensor(f"xt{p}", [128, XW], f32r))
            for p in range(BPC)
        ]
        osbs = [
            ctx.enter_context(nc.sbuf_tensor(f"osb{p}", [128, OW], f32))
            for p in range(BPC)
        ]
        psums = [
            ctx.enter_context(nc.psum_tensor(f"ps{j}", [128, 512], f32))
            for j in range(NJ)
        ]
        sem_w = ctx.enter_context(nc.semaphore("sem_w"))
        sem_xs = [
            ctx.enter_context(nc.semaphore(f"sem_x{p}")) for p in range(BPC)
        ]
        sem_mm = ctx.enter_context(nc.semaphore("sem_mm"))
        sem_cpa = ctx.enter_context(nc.semaphore("sem_cpa"))
        sem_cpb = ctx.enter_context(nc.semaphore("sem_cpb"))
        sem_outs = [
            ctx.enter_context(nc.semaphore(f"sem_out{p}")) for p in range(BPC)
        ]
        block = ctx.enter_context(nc.Block())

        NPAIR = BPC * reps  # global pair count

        @block.sync
        def _(sync):
            sync.dma_start(out=wt[:], in_=w[:, :]).then_inc(sem_w, 16)
            # With act_out, outs issue from the ACT sequencer onto the
            # second HWDGE ring (qActDynamicHW): in- and out-streams run
            # on separate rings and SP has no in/out FIFO coupling.
            # Otherwise interleave outs one pair behind ins on SP
            # (issuing all ins first deadlocks at high reps).
            for gp in range(NPAIR + 1):
                if gp < NPAIR:
                    p = gp % BPC
                    r = gp // BPC
                    if r > 0:
                        # xt slot reuse: previous rep's reads of this
                        # pair done once its 2nd tile's groups complete
                        sync.wait_ge(
                            sem_mm, NJ * ((r - 1) * NTILES + 2 * p + 2)
                        )
                    sync.dma_start(out=xts[p][:], in_=x[p, :, :]).then_inc(
                        sem_xs[p], 16
                    )
                op = gp - 1
                if op >= 0 and not act_out:
                    p = op % BPC
                    r = op // BPC
                    gt_a = r * NTILES + 2 * p      # even tile -> ACT
                    gt_b = gt_a + 1                # odd tile -> DVE
                    sync.wait_ge(sem_cpa, _cp_sem_count(gt_a, NJ - 1))
                    sync.wait_ge(sem_cpb, _cp_sem_count(gt_b, NJ - 1))
                    sync.dma_start(out=out[p, :, :], in_=osbs[p][:]).then_inc(
                        sem_outs[p], 16
                    )
            for p in range(BPC):
                sync.wait_ge(sem_outs[p], 16 * reps)

        @block.tensor
        def _(tensor):
            tensor.wait_ge(sem_w, 16)
            for r in range(reps):
                for ti in range(NTILES):
                    gt = r * NTILES + ti
                    p, u = divmod(ti, NT)
                    if u == 0:
                        tensor.wait_ge(sem_xs[p], 16 * (r + 1))
                    xbase = u * (G + 2)
                    for j in range(NJ):
                        if gt > 0:
                            # PSUM bank j drained by the previous global
                            # tile's engine
                            prev_sem = (
                                sem_cpa if (gt - 1) % 2 == 0 else sem_cpb
                            )
                            tensor.wait_ge(prev_sem, _cp_sem_count(gt - 1, j))
                        mm = None
                        for k in range(KT):
                            a = xbase + j * 512 + k
                            mm = tensor.matmul(
                                psums[j][:],
                                wt[:, k * 128 : (k + 1) * 128],
                                xts[p][:, a : a + 512],
                                start=(k == 0),
                                stop=(k == KT - 1),
                            )
                        mm.then_inc(sem_mm, 1)

        @block.scalar
        def _(scalar):
            for gt in range(0, NTILES * reps, 2):
                ti = gt % NTILES
                p, u = divmod(ti, NT)
                obase = u * G
                for j in range(NJ):
                    scalar.wait_ge(sem_mm, gt * NJ + j + 1)
                    if gt >= NTILES and j == 0:
                        # osb slot reuse: previous rep's out-DMA done
                        scalar.wait_ge(sem_outs[p], 16 * (gt // NTILES))
                    scalar.copy(
                        osbs[p][:, obase + j * 512 : obase + (j + 1) * 512],
                        psums[j][:],
                    ).then_inc(sem_cpa, 1)
                if act_out:
                    # ACT just drained tile u=0 of pair p (FIFO order);
                    # wait for DVE's u=1 tile, then issue the pair's
                    # out-DMA on the ACT HWDGE ring.
                    scalar.wait_ge(sem_cpb, _cp_sem_count(gt + 1, NJ - 1))
                    scalar.dma_start(
                        out=out[p, :, :], in_=osbs[p][:]
                    ).then_inc(sem_outs[p], 16)

        @block.vector
        def _(vector):
            for gt in range(1, NTILES * reps, 2):
                ti = gt % NTILES
                p, u = divmod(ti, NT)
                obase = u * G
                for j in range(NJ):
                    vector.wait_ge(sem_mm, gt * NJ + j + 1)
                    if gt >= NTILES and j == 0:
                        vector.wait_ge(sem_outs[p], 16 * (gt // NTILES))
                    vector.tensor_copy(
                        osbs[p][:, obase + j * 512 : obase + (j + 1) * 512],
                        psums[j][:],
                    ).then_inc(sem_cpb, 1)

    return nc


def _block_diag_weights(kernel: np.ndarray) -> np.ndarray:
    """kernel (1, CO, C, KT) -> (128, KT*128) block-diag lhsT, SBUF layout.

    row (ci + 32*g), col (k*128 + co + 32*g) = kernel[0, co, ci, k]
    """
    wbd = np.zeros((KT, 128, 128), dtype=np.float32)
    wt = np.ascontiguousarray(kernel[0].transpose(2, 1, 0))  # (KT, C, CO)
    for g in range(NG):
        wbd[:, g * 32 : (g + 1) * 32, g * 32 : (g + 1) * 32] = wt
    return np.ascontiguousarray(wbd.transpose(1, 0, 2)).reshape(128, KT * 128)


def _pack_x(x: np.ndarray) -> np.ndarray:
    """(B, C, L) -> (NCORES, BPC, 128, XW) packed, padded by 2.

    Row (g*32 + ci), col (t*(G+2) + j) of batch b's block holds
    x[b, ci, t*TILE_L + g*G + j] (zeros past L).
    """
    xp = np.zeros((B, C, LP), dtype=np.float32)
    xp[:, :, :L] = x
    sb, sc, sl = (s // 4 for s in xp.strides)
    win = np.lib.stride_tricks.as_strided(
        xp,
        shape=(B, NG, C, NT, G + 2),
        strides=tuple(4 * s for s in (sb, G, sc, TILE_L, sl)),
    )
    return np.ascontiguousarray(win).reshape(NCORES, BPC, 128, XW)


def _unpack_out(packed: np.ndarray) -> np.ndarray:
    """(NCORES, BPC, 128, OW) -> (B, CO, LOUT)."""
    arr = packed.reshape(NCORES, BPC, NG, CO, NT, G)
    arr = arr.transpose(0, 1, 3, 4, 2, 5)  # core, b, co, t, g, j
    return np.ascontiguousarray(arr).reshape(B, CO, LOP)[:, :, :LOUT]


def kernel(x: np.ndarray, kernel: np.ndarray) -> np.ndarray:
    if "nc" not in _CACHE:
        _CACHE["nc"] = _build_nc()
    nc = _CACHE["nc"]

    wbd = _block_diag_weights(np.asarray(kernel, dtype=np.float32))
    xpk = _pack_x(np.asarray(x, dtype=np.float32))

    in_maps = [{"x": xpk[i], "w": wbd} for i in range(NCORES)]
    res = run_bass_kernel_spmd(nc, in_maps, list(range(NCORES)))
    packed = np.stack([r["out"] for r in res.results], axis=0)
    return _unpack_out(packed)


# revision 25
# speedup vs baseline: 1.0260x; 1.0260x over previous
"""Strided (stride=1) valid 1D conv on Trainium2, data-parallel over batch.

Problem: x (16, 32, 32768) f32, kernel (1, 32, 32, 3) f32
         -> out (16, 32, 32766) f32  (valid conv, NCH / OIH layout)

Strategy (per core, 2 batches each across 8 cores):
  out[b, co, l] = sum_{ci,k} W[co, ci, k] * x[b, ci, l + k]

  Channel count is 32, so we pack 4 independent L-chunks ("groups") into
  the 128 SBUF partitions: partition (g*32 + ci) holds x[b, ci, base+g*G+j].
  A block-diagonal [128, 128] weight matrix (4 copies of W_k^T on the
  diagonal) turns the 4-group conv tap into ONE K=128 matmul.  The 3 taps
  accumulate into one PSUM bank (start/stop flags).  Operands are typed
  float32r for the fast PE path (1 cycle/row at N=512 vs 4 for fp32).

  The host pre-packs x into the exact SBUF layout and unpacks the packed
  output, so every device DMA is one contiguous stream per partition row.
  Measured on HW: contiguous descriptors ~350-380 GB/s/core vs ~130 GB/s
  for 3D-strided ones; 4.2 MB transfers beat 2.1 MB by ~7%.  One in-DMA
  and one out-DMA per local batch ("pair" of two 16K-column tiles).

  Raw Bass (not Tile): walrus codegen in this toolchain embeds at most
  ONE sync wait per Matmult / HWDGE DMACopy, which Tile's auto-generated
  semaphores routinely exceed.  Every cross-engine wait here is an
  explicit standalone wait_ge on the engine's sequencer:
    sync   : weight DMA, then per-pair input DMAs interleaved with
             output DMAs (lagged one pair).
    tensor : per tile, 8 groups x 3 accumulating fp32r matmuls; PSUM
             bank j is recycled across tiles, gated on the drain of its
             previous occupant; last matmul of a group bumps sem_mm.
    scalar : drains PSUM->SBUF for even global tiles (ACT engine).
    vector : drains PSUM->SBUF for odd global tiles (DVE engine).

  reps > 1 repeats the whole pipeline in one NEFF (benchmarking only).
"""

import sys

if "/opt/trn_rl_repo" not in sys.path:
    sys.path.insert(0, "/opt/trn_rl_repo")

from contextlib import ExitStack

import numpy as np

import concourse.bass as bass
import concourse.mybir as mybir
from concourse.bass_utils import run_bass_kernel_spmd

# Problem shape (hardcoded; harness contract)
B, C, L = 16, 32, 32768
CO, KT = 32, 3
LOUT = L - KT + 1  # 32766
NCORES = 8
BPC = B // NCORES  # batches per core = 2

# Padded shapes
LP = L + 2  # x padded with 2 trailing zero columns
LOP = L     # output computed padded to 32768 (last 2 cols garbage)

# Tiling
NG = 4              # L-groups packed across the 128 partitions
G = 4096            # columns per group per tile
TILE_L = NG * G     # 16384 output cols per tile
NT = LOP // TILE_L  # tiles per batch = 2
NJ = G // 512       # 512-wide matmul chunks per group = 8
NTILES = BPC * NT   # 4 tiles/core; pair p (= local batch) owns tiles 2p,2p+1
XW = NT * (G + 2)   # xt pair-row width
OW = NT * G         # osb pair-row width

_CACHE = {}


def _cp_sem_count(gt: int, j: int) -> int:
    """Drain-engine sem value after copy (gt, j) completes.

    ACT drains even global tiles, DVE odd ones; each engine's sem counts
    its own copies in order.  gt = rep * NTILES + ti.
    """
    return NJ * (gt // 2) + j + 1


def _build_nc(reps: int = 1, act_out: bool = False, split_ends: bool = True):
    f32 = mybir.dt.float32
    f32r = mybir.dt.float32r

    nc = bass.Bass(trn_type="TRN2", target_bir_lowering=False)
    x = nc.dram_tensor("x", [BPC, 128, XW], f32r, kind="ExternalInput")
    w = nc.dram_tensor("w", [128, KT * 128], f32r, kind="ExternalInput")
    out = nc.dram_tensor("out", [BPC, 128, OW], f32, kind="ExternalOutput")

    with ExitStack() as ctx:
        wt = ctx.enter_context(nc.sbuf_tensor("wt", [128, KT * 128], f32r))
        xts = [
            ctx.enter_context(nc.sbuf_tensor(f"xt{p}", [128, XW], f32r))
            for p in range(BPC)
        ]
        osbs = [
            ctx.enter_context(nc.sbuf_tensor(f"osb{p}", [128, OW], f32))
            for p in range(BPC)
        ]
        psums = [
            ctx.enter_context(nc.psum_tensor(f"ps{j}", [128, 512], f32))
            for j in range(NJ)
        ]
        sem_w = ctx.enter_context(nc.semaphore("sem_w"))
        sem_xs = [
            ctx.enter_context(nc.semaphore(f"sem_x{p}")) for p in range(BPC)
        ]
        sem_mm = ctx.enter_context(nc.semaphore("sem_mm"))
        sem_cpa = ctx.enter_context(nc.semaphore("sem_cpa"))
        sem_cpb = ctx.enter_context(nc.semaphore("sem_cpb"))
        sem_outs = [
            ctx.enter_context(nc.semaphore(f"sem_out{p}")) for p in range(BPC)
        ]
        # second-half sems for the split boundary DMAs (a counting sem
        # shared by concurrently in-flight DMAs is unsound)
        sem_xt = ctx.enter_context(nc.semaphore("sem_xt"))
        sem_ot = ctx.enter_context(nc.semaphore("sem_ot"))
        block = ctx.enter_context(nc.Block())

        NPAIR = BPC * reps  # global pair count

        @block.sync
        def _(sync):
            sync.dma_start(out=wt[:], in_=w[:, :]).then_inc(sem_w, 16)
            # With act_out, outs issue from the ACT sequencer onto the
            # second HWDGE ring (qActDynamicHW): in- and out-streams run
            # on separate rings and SP has no in/out FIFO coupling.
            # Otherwise interleave outs one pair behind ins on SP
            # (issuing all ins first deadlocks at high reps).
            H = G + 2
            for gp in range(NPAIR + 1):
                if gp < NPAIR:
                    p = gp % BPC
                    r = gp // BPC
                    if r > 0:
                        # xt slot reuse: previous rep's reads of this
                        # pair done once its 2nd tile's groups complete
                        sync.wait_ge(
                            sem_mm, NJ * ((r - 1) * NTILES + 2 * p + 2)
                        )
                    if split_ends and gp == 0:
                        # fill latency: land tile 0's half first so PE
                        # starts ~halfway through the pair transfer
                        sync.dma_start(
                            out=xts[p][:, 0:H], in_=x[p, :, 0:H]
                        ).then_inc(sem_xs[p], 16)
                        sync.dma_start(
                            out=xts[p][:, H:XW], in_=x[p, :, H:XW]
                        ).then_inc(sem_xt, 16)
                    else:
                        sync.dma_start(
                            out=xts[p][:], in_=x[p, :, :]
                        ).then_inc(sem_xs[p], 16)
                op = gp - 1
                if op >= 0 and not act_out:
                    p = op % BPC
                    r = op // BPC
                    gt_a = r * NTILES + 2 * p      # even tile -> ACT
                    gt_b = gt_a + 1                # odd tile -> DVE
                    if split_ends and op == NPAIR - 1:
                        # drain latency: ship tile u=0's half as soon as
                        # ACT drained it; u=1 half after DVE finishes
                        sync.wait_ge(sem_cpa, _cp_sem_count(gt_a, NJ - 1))
                        sync.dma_start(
                            out=out[p, :, 0:G], in_=osbs[p][:, 0:G]
                        ).then_inc(sem_outs[p], 16)
                        sync.wait_ge(sem_cpb, _cp_sem_count(gt_b, NJ - 1))
                        sync.dma_start(
                            out=out[p, :, G:OW], in_=osbs[p][:, G:OW]
                        ).then_inc(sem_ot, 16)
                    else:
                        sync.wait_ge(sem_cpa, _cp_sem_count(gt_a, NJ - 1))
                        sync.wait_ge(sem_cpb, _cp_sem_count(gt_b, NJ - 1))
                        sync.dma_start(
                            out=out[p, :, :], in_=osbs[p][:]
                        ).then_inc(sem_outs[p], 16)
            for p in range(BPC):
                sync.wait_ge(sem_outs[p], 16 * reps)
            if split_ends:
                sync.wait_ge(sem_ot, 16)

        @block.tensor
        def _(tensor):
            tensor.wait_ge(sem_w, 16)
            for r in range(reps):
                for ti in range(NTILES):
                    gt = r * NTILES + ti
                    p, u = divmod(ti, NT)
                    if split_ends and r == 0 and p == 0:
                        # pair 0 rep 0 arrives as two halves; tile u only
                        # reads its own half
                        if u == 0:
                            tensor.wait_ge(sem_xs[p], 16)
                        else:
                            tensor.wait_ge(sem_xt, 16)
                    elif u == 0:
                        tensor.wait_ge(sem_xs[p], 16 * (r + 1))
                    xbase = u * (G + 2)
                    for j in range(NJ):
                        if gt > 0:
                            # PSUM bank j drained by the previous global
                            # tile's engine
                            prev_sem = (
                                sem_cpa if (gt - 1) % 2 == 0 else sem_cpb
                            )
                            tensor.wait_ge(prev_sem, _cp_sem_count(gt - 1, j))
                        mm = None
                        for k in range(KT):
                            a = xbase + j * 512 + k
                            mm = tensor.matmul(
                                psums[j][:],
                                wt[:, k * 128 : (k + 1) * 128],
                                xts[p][:, a : a + 512],
                                start=(k == 0),
                                stop=(k == KT - 1),
                            )
                        mm.then_inc(sem_mm, 1)

        @block.scalar
        def _(scalar):
            for gt in range(0, NTILES * reps, 2):
                ti = gt % NTILES
                p, u = divmod(ti, NT)
                obase = u * G
                for j in range(NJ):
                    scalar.wait_ge(sem_mm, gt * NJ + j + 1)
                    if gt >= NTILES and j == 0:
                        # osb slot reuse: previous rep's out-DMA done
                        scalar.wait_ge(sem_outs[p], 16 * (gt // NTILES))
                    scalar.copy(
                        osbs[p][:, obase + j * 512 : obase + (j + 1) * 512],
                        psums[j][:],
                    ).then_inc(sem_cpa, 1)
                if act_out:
                    # ACT just drained tile u=0 of pair p (FIFO order);
                    # wait for DVE's u=1 tile, then issue the pair's
                    # out-DMA on the ACT HWDGE ring.
                    scalar.wait_ge(sem_cpb, _cp_sem_count(gt + 1, NJ - 1))
                    scalar.dma_start(
                        out=out[p, :, :], in_=osbs[p][:]
                    ).then_inc(sem_outs[p], 16)

        @block.vector
        def _(vector):
            for gt in range(1, NTILES * reps, 2):
                ti = gt % NTILES
                p, u = divmod(ti, NT)
                obase = u * G
                for j in range(NJ):
                    vector.wait_ge(sem_mm, gt * NJ + j + 1)
                    if gt >= NTILES and j == 0:
                        vector.wait_ge(sem_outs[p], 16 * (gt // NTILES))
                    vector.tensor_copy(
                        osbs[p][:, obase + j * 512 : obase + (j + 1) * 512],
                        psums[j][:],
                    ).then_inc(sem_cpb, 1)

    return nc


def _block_diag_weights(kernel: np.ndarray) -> np.ndarray:
    """kernel (1, CO, C, KT) -> (128, KT*128) block-diag lhsT, SBUF layout.

    row (ci + 32*g), col (k*128 + co + 32*g) = kernel[0, co, ci, k]
    """
    wbd = np.zeros((KT, 128, 128), dtype=np.float32)
    wt = np.ascontiguousarray(kernel[0].transpose(2, 1, 0))  # (KT, C, CO)
    for g in range(NG):
        wbd[:, g * 32 : (g + 1) * 32, g * 32 : (g + 1) * 32] = wt
    return np.ascontiguousarray(wbd.transpose(1, 0, 2)).reshape(128, KT * 128)


def _pack_x(x: np.ndarray) -> np.ndarray:
    """(B, C, L) -> (NCORES, BPC, 128, XW) packed, padded by 2.

    Row (g*32 + ci), col (t*(G+2) + j) of batch b's block holds
    x[b, ci, t*TILE_L + g*G + j] (zeros past L).
    """
    xp = np.zeros((B, C, LP), dtype=np.float32)
    xp[:, :, :L] = x
    sb, sc, sl = (s // 4 for s in xp.strides)
    win = np.lib.stride_tricks.as_strided(
        xp,
        shape=(B, NG, C, NT, G + 2),
        strides=tuple(4 * s for s in (sb, G, sc, TILE_L, sl)),
    )
    return np.ascontiguousarray(win).reshape(NCORES, BPC, 128, XW)


def _unpack_out(packed: np.ndarray) -> np.ndarray:
    """(NCORES, BPC, 128, OW) -> (B, CO, LOUT)."""
    arr = packed.reshape(NCORES, BPC, NG, CO, NT, G)
    arr = arr.transpose(0, 1, 3, 4, 2, 5)  # core, b, co, t, g, j
    return np.ascontiguousarray(arr).reshape(B, CO, LOP)[:, :, :LOUT]


def kernel(x: np.ndarray, kernel: np.ndarray) -> np.ndarray:
    if "nc" not in _CACHE:
        _CACHE["nc"] = _build_nc()
    nc = _CACHE["nc"]

    wbd = _block_diag_weights(np.asarray(kernel, dtype=np.float32))
    xpk = _pack_x(np.asarray(x, dtype=np.float32))

    in_maps = [{"x": xpk[i], "w": wbd} for i in range(NCORES)]
    res = run_bass_kernel_spmd(nc, in_maps, list(range(NCORES)))
    packed = np.stack([r["out"] for r in res.results], axis=0)
    return _unpack_out(packed)


# revision 26
# speedup vs baseline: 1.1216x; 1.0931x over previous
"""Strided (stride=1) valid 1D conv on Trainium2, data-parallel over batch.

Problem: x (16, 32, 32768) f32, kernel (1, 32, 32, 3) f32
         -> out (16, 32, 32766) f32  (valid conv, NCH / OIH layout)

Strategy (per core, 2 batches each across 8 cores):
  out[b, co, l] = sum_{ci,k} W[co, ci, k] * x[b, ci, l + k]

  Channel count is 32, so we pack 4 independent L-chunks ("groups") into
  the 128 SBUF partitions: partition (g*32 + ci) holds x[b, ci, base+g*G+j].
  A block-diagonal [128, 128] weight matrix (4 copies of W_k^T on the
  diagonal) turns the 4-group conv tap into ONE K=128 matmul.  The 3 taps
  accumulate into one PSUM bank (start/stop flags).  Operands are typed
  float32r for the fast PE path (1 cycle/row at N=512 vs 4 for fp32).

  The host pre-packs x into the exact SBUF layout and unpacks the packed
  output, so every device DMA is one contiguous stream per partition row.
  Measured on HW: contiguous descriptors ~350-380 GB/s/core vs ~130 GB/s
  for 3D-strided ones; 4.2 MB transfers beat 2.1 MB by ~7%.  One in-DMA
  and one out-DMA per local batch ("pair" of two 16K-column tiles).

  Raw Bass (not Tile): walrus codegen in this toolchain embeds at most
  ONE sync wait per Matmult / HWDGE DMACopy, which Tile's auto-generated
  semaphores routinely exceed.  Every cross-engine wait here is an
  explicit standalone wait_ge on the engine's sequencer:
    sync   : weight DMA, then per-pair input DMAs interleaved with
             output DMAs (lagged one pair).
    tensor : per tile, 8 groups x 3 accumulating fp32r matmuls; PSUM
             bank j is recycled across tiles, gated on the drain of its
             previous occupant; last matmul of a group bumps sem_mm.
    scalar : drains PSUM->SBUF for even global tiles (ACT engine).
    vector : drains PSUM->SBUF for odd global tiles (DVE engine).

  reps > 1 repeats the whole pipeline in one NEFF (benchmarking only).
"""

import sys

if "/opt/trn_rl_repo" not in sys.path:
    sys.path.insert(0, "/opt/trn_rl_repo")

from contextlib import ExitStack

import numpy as np

import concourse.bass as bass
import concourse.mybir as mybir
from concourse.bass_utils import run_bass_kernel_spmd

# Problem shape (hardcoded; harness contract)
B, C, L = 16, 32, 32768
CO, KT = 32, 3
LOUT = L - KT + 1  # 32766
NCORES = 8
BPC = B // NCORES  # batches per core = 2

# Padded shapes
LP = L + 2  # x padded with 2 trailing zero columns
LOP = L     # output computed padded to 32768 (last 2 cols garbage)

# Tiling
NG = 4              # L-groups packed across the 128 partitions
G = 4096            # columns per group per tile
TILE_L = NG * G     # 16384 output cols per tile
NT = LOP // TILE_L  # tiles per batch = 2
NJ = G // 512       # 512-wide matmul chunks per group = 8
NTILES = BPC * NT   # 4 tiles/core; pair p (= local batch) owns tiles 2p,2p+1
XW = NT * (G + 2)   # xt pair-row width
OW = NT * G         # osb pair-row width

_CACHE = {}


def _cp_sem_count(gt: int, j: int) -> int:
    """Drain-engine sem value after copy (gt, j) completes.

    ACT drains even global tiles, DVE odd ones; each engine's sem counts
    its own copies in order.  gt = rep * NTILES + ti.
    """
    return NJ * (gt // 2) + j + 1


def _build_nc(reps: int = 1, act_out: bool = False, split_ends: bool = True):
    f32 = mybir.dt.float32
    f32r = mybir.dt.float32r

    nc = bass.Bass(trn_type="TRN2", target_bir_lowering=False)
    x = nc.dram_tensor("x", [BPC, 128, XW], f32r, kind="ExternalInput")
    w = nc.dram_tensor("w", [128, KT * 128], f32r, kind="ExternalInput")
    out = nc.dram_tensor("out", [BPC, 128, OW], f32, kind="ExternalOutput")

    with ExitStack() as ctx:
        wt = ctx.enter_context(nc.sbuf_tensor("wt", [128, KT * 128], f32r))
        xts = [
            ctx.enter_context(nc.sbuf_tensor(f"xt{p}", [128, XW], f32r))
            for p in range(BPC)
        ]
        osbs = [
            ctx.enter_context(nc.sbuf_tensor(f"osb{p}", [128, OW], f32))
            for p in range(BPC)
        ]
        psums = [
            ctx.enter_context(nc.psum_tensor(f"ps{j}", [128, 512], f32))
            for j in range(NJ)
        ]
        sem_w = ctx.enter_context(nc.semaphore("sem_w"))
        sem_xs = [
            ctx.enter_context(nc.semaphore(f"sem_x{p}")) for p in range(BPC)
        ]
        sem_mm = ctx.enter_context(nc.semaphore("sem_mm"))
        sem_cpa = ctx.enter_context(nc.semaphore("sem_cpa"))
        sem_cpb = ctx.enter_context(nc.semaphore("sem_cpb"))
        sem_outs = [
            ctx.enter_context(nc.semaphore(f"sem_out{p}")) for p in range(BPC)
        ]
        # fragment sems for the split boundary DMAs (a counting sem
        # shared by concurrently in-flight DMAs is unsound)
        sem_xt = ctx.enter_context(nc.semaphore("sem_xt"))
        sem_xq = ctx.enter_context(nc.semaphore("sem_xq"))
        sem_ot = ctx.enter_context(nc.semaphore("sem_ot"))
        sem_oq = ctx.enter_context(nc.semaphore("sem_oq"))
        block = ctx.enter_context(nc.Block())

        NPAIR = BPC * reps  # global pair count

        @block.sync
        def _(sync):
            sync.dma_start(out=wt[:], in_=w[:, :]).then_inc(sem_w, 16)
            # With act_out, outs issue from the ACT sequencer onto the
            # second HWDGE ring (qActDynamicHW): in- and out-streams run
            # on separate rings and SP has no in/out FIFO coupling.
            # Otherwise interleave outs one pair behind ins on SP
            # (issuing all ins first deadlocks at high reps).
            H = G + 2
            for gp in range(NPAIR + 1):
                if gp < NPAIR:
                    p = gp % BPC
                    r = gp // BPC
                    if r > 0:
                        # xt slot reuse: previous rep's reads of this
                        # pair done once its 2nd tile's groups complete
                        sync.wait_ge(
                            sem_mm, NJ * ((r - 1) * NTILES + 2 * p + 2)
                        )
                    if split_ends and gp == 0:
                        # fill latency: land tile 0's first quarter
                        # (covers matmul chunks j<4, which read cols
                        # [0, 2050)) so PE starts ~1/4 into the pair
                        # transfer; then the rest of tile 0, then tile 1
                        Q = (NJ // 2 - 1) * 512 + 514  # 2050
                        sync.dma_start(
                            out=xts[p][:, 0:Q], in_=x[p, :, 0:Q]
                        ).then_inc(sem_xs[p], 16)
                        sync.dma_start(
                            out=xts[p][:, Q:H], in_=x[p, :, Q:H]
                        ).then_inc(sem_xq, 16)
                        sync.dma_start(
                            out=xts[p][:, H:XW], in_=x[p, :, H:XW]
                        ).then_inc(sem_xt, 16)
                    else:
                        sync.dma_start(
                            out=xts[p][:], in_=x[p, :, :]
                        ).then_inc(sem_xs[p], 16)
                op = gp - 1
                if op >= 0 and not act_out:
                    p = op % BPC
                    r = op // BPC
                    gt_a = r * NTILES + 2 * p      # even tile -> ACT
                    gt_b = gt_a + 1                # odd tile -> DVE
                    if split_ends and op == NPAIR - 1:
                        # drain latency: ship tile u=0's half as soon as
                        # ACT drained it; u=1 half after DVE finishes
                        sync.wait_ge(sem_cpa, _cp_sem_count(gt_a, NJ - 1))
                        sync.dma_start(
                            out=out[p, :, 0:G], in_=osbs[p][:, 0:G]
                        ).then_inc(sem_outs[p], 16)
                        M = G + (NJ // 2) * 512  # B-tile midpoint
                        sync.wait_ge(sem_cpb, _cp_sem_count(gt_b, NJ // 2 - 1))
                        sync.dma_start(
                            out=out[p, :, G:M], in_=osbs[p][:, G:M]
                        ).then_inc(sem_oq, 16)
                        sync.wait_ge(sem_cpb, _cp_sem_count(gt_b, NJ - 1))
                        sync.dma_start(
                            out=out[p, :, M:OW], in_=osbs[p][:, M:OW]
                        ).then_inc(sem_ot, 16)
                    else:
                        sync.wait_ge(sem_cpa, _cp_sem_count(gt_a, NJ - 1))
                        sync.wait_ge(sem_cpb, _cp_sem_count(gt_b, NJ - 1))
                        sync.dma_start(
                            out=out[p, :, :], in_=osbs[p][:]
                        ).then_inc(sem_outs[p], 16)
            for p in range(BPC):
                sync.wait_ge(sem_outs[p], 16 * reps)
            if split_ends:
                sync.wait_ge(sem_oq, 16)
                sync.wait_ge(sem_ot, 16)

        @block.tensor
        def _(tensor):
            tensor.wait_ge(sem_w, 16)
            for r in range(reps):
                for ti in range(NTILES):
                    gt = r * NTILES + ti
                    p, u = divmod(ti, NT)
                    if split_ends and r == 0 and p == 0:
                        # pair 0 rep 0 arrives in fragments; tile u only
                        # reads its own fragment(s)
                        if u == 0:
                            tensor.wait_ge(sem_xs[p], 16)
                        else:
                            tensor.wait_ge(sem_xt, 16)
                    elif u == 0:
                        tensor.wait_ge(sem_xs[p], 16 * (r + 1))
                    xbase = u * (G + 2)
                    for j in range(NJ):
                        if split_ends and gt == 0 and u == 0 and j == NJ // 2:
                            # 2nd quarter of the first fill fragment set
                            tensor.wait_ge(sem_xq, 16)
                        if gt > 0:
                            # PSUM bank j drained by the previous global
                            # tile's engine
                            prev_sem = (
                                sem_cpa if (gt - 1) % 2 == 0 else sem_cpb
                            )
                            tensor.wait_ge(prev_sem, _cp_sem_count(gt - 1, j))
                        mm = None
                        for k in range(KT):
                            a = xbase + j * 512 + k
                            mm = tensor.matmul(
                                psums[j][:],
                                wt[:, k * 128 : (k + 1) * 128],
                                xts[p][:, a : a + 512],
                                start=(k == 0),
                                stop=(k == KT - 1),
                            )
                        mm.then_inc(sem_mm, 1)

        @block.scalar
        def _(scalar):
            for gt in range(0, NTILES * reps, 2):
                ti = gt % NTILES
                p, u = divmod(ti, NT)
                obase = u * G
                for j in range(NJ):
                    scalar.wait_ge(sem_mm, gt * NJ + j + 1)
                    if gt >= NTILES and j == 0:
                        # osb slot reuse: previous rep's out-DMA done
                        scalar.wait_ge(sem_outs[p], 16 * (gt // NTILES))
                    scalar.copy(
                        osbs[p][:, obase + j * 512 : obase + (j + 1) * 512],
                        psums[j][:],
                    ).then_inc(sem_cpa, 1)
                if act_out:
                    # ACT just drained tile u=0 of pair p (FIFO order);
                    # wait for DVE's u=1 tile, then issue the pair's
                    # out-DMA on the ACT HWDGE ring.
                    scalar.wait_ge(sem_cpb, _cp_sem_count(gt + 1, NJ - 1))
                    scalar.dma_start(
                        out=out[p, :, :], in_=osbs[p][:]
                    ).then_inc(sem_outs[p], 16)

        @block.vector
        def _(vector):
            for gt in range(1, NTILES * reps, 2):
                ti = gt % NTILES
                p, u = divmod(ti, NT)
                obase = u * G
                for j in range(NJ):
                    vector.wait_ge(sem_mm, gt * NJ + j + 1)
                    if gt >= NTILES and j == 0:
                        vector.wait_ge(sem_outs[p], 16 * (gt // NTILES))
                    vector.tensor_copy(
                        osbs[p][:, obase + j * 512 : obase + (j + 1) * 512],
                        psums[j][:],
                    ).then_inc(sem_cpb, 1)

    return nc


def _block_diag_weights(kernel: np.ndarray) -> np.ndarray:
    """kernel (1, CO, C, KT) -> (128, KT*128) block-diag lhsT, SBUF layout.

    row (ci + 32*g), col (k*128 + co + 32*g) = kernel[0, co, ci, k]
    """
    wbd = np.zeros((KT, 128, 128), dtype=np.float32)
    wt = np.ascontiguousarray(kernel[0].transpose(2, 1, 0))  # (KT, C, CO)
    for g in range(NG):
        wbd[:, g * 32 : (g + 1) * 32, g * 32 : (g + 1) * 32] = wt
    return np.ascontiguousarray(wbd.transpose(1, 0, 2)).reshape(128, KT * 128)


def _pack_x(x: np.ndarray) -> np.ndarray:
    """(B, C, L) -> (NCORES, BPC, 128, XW) packed, padded by 2.

    Row (g*32 + ci), col (t*(G+2) + j) of batch b's block holds
    x[b, ci, t*TILE_L + g*G + j] (zeros past L).
    """
    xp = np.zeros((B, C, LP), dtype=np.float32)
    xp[:, :, :L] = x
    sb, sc, sl = (s // 4 for s in xp.strides)
    win = np.lib.stride_tricks.as_strided(
        xp,
        shape=(B, NG, C, NT, G + 2),
        strides=tuple(4 * s for s in (sb, G, sc, TILE_L, sl)),
    )
    return np.ascontiguousarray(win).reshape(NCORES, BPC, 128, XW)


def _unpack_out(packed: np.ndarray) -> np.ndarray:
    """(NCORES, BPC, 128, OW) -> (B, CO, LOUT)."""
    arr = packed.reshape(NCORES, BPC, NG, CO, NT, G)
    arr = arr.transpose(0, 1, 3, 4, 2, 5)  # core, b, co, t, g, j
    return np.ascontiguousarray(arr).reshape(B, CO, LOP)[:, :, :LOUT]


def kernel(x: np.ndarray, kernel: np.ndarray) -> np.ndarray:
    if "nc" not in _CACHE:
        _CACHE["nc"] = _build_nc()
    nc = _CACHE["nc"]

    wbd = _block_diag_weights(np.asarray(kernel, dtype=np.float32))
    xpk = _pack_x(np.asarray(x, dtype=np.float32))

    in_maps = [{"x": xpk[i], "w": wbd} for i in range(NCORES)]
    res = run_bass_kernel_spmd(nc, in_maps, list(range(NCORES)))
    packed = np.stack([r["out"] for r in res.results], axis=0)
    return _unpack_out(packed)
